# revision 5
# baseline (speedup 1.0000x reference)
"""Multi-head causal attention (B=2, S=2048, E=1024, H=16) on 8 NeuronCores.

Sharding: one core per (batch, head-group-of-4). Each core:
  - computes Q/K/V projections for its 256 embed dims (bf16 matmuls, fp32 psum)
  - causal attention for its 4 heads (scores transposed [t,s]; softmax
    denominators via a ones-row appended to V; exp on ScalarE from PSUM)
  - partial output projection y_part = attnout @ Wo_cols^T
Host sums the 4 partials per batch and adds bo.
"""

import numpy as np
import ml_dtypes
from contextlib import ExitStack

import concourse.bacc as bacc
import concourse.mybir as mybir
import concourse.tile as tile

BF16 = mybir.dt.bfloat16
F32 = mybir.dt.float32
AF = mybir.ActivationFunctionType
ALU = mybir.AluOpType

EMBED = 1024
KO = EMBED // 128  # 8 contraction tiles over embed
N_CORES = 8
SCALE = 0.125  # 1/sqrt(64)


def build_nc(S):
    """Emit the per-core kernel. Identical program on all cores (SPMD)."""
    NSB = S // 512  # 512-wide s blocks
    NTT = S // 128  # 128-wide s/t tiles

    nc = bacc.Bacc("TRN2", target_bir_lowering=False)

    xT_d = nc.dram_tensor("xT", [128, KO, S], BF16, kind="ExternalInput")
    wqT_d = nc.dram_tensor("wqT", [128, KO, 256], BF16, kind="ExternalInput")
    wkT_d = nc.dram_tensor("wkT", [128, KO, 256], BF16, kind="ExternalInput")
    wvT_d = nc.dram_tensor("wvT", [128, KO, 256], BF16, kind="ExternalInput")
    woT_d = nc.dram_tensor("woT", [128, 2, EMBED], BF16, kind="ExternalInput")
    bq_d = nc.dram_tensor("bq2", [128, 2], F32, kind="ExternalInput")
    bk_d = nc.dram_tensor("bk2", [128, 2], F32, kind="ExternalInput")
    bv_d = nc.dram_tensor("bvb", [128, 256], F32, kind="ExternalInput")
    mk_d = nc.dram_tensor("maskM", [128, 512], BF16, kind="ExternalInput")
    y_d = nc.dram_tensor("y", [S, EMBED], F32, kind="ExternalOutput")
    yv = y_d[:].rearrange("(so p) e -> so p e", p=128)

    with tile.TileContext(nc) as tc, ExitStack() as ctx:
        consts = ctx.enter_context(tc.tile_pool(name="consts", bufs=1))

        xT = consts.tile([128, KO, S], BF16, name="xTs")
        for k in range(KO):
            nc.sync.dma_start(xT[:, k, :], xT_d[:, k, :])
        wqT = consts.tile([128, KO, 256], BF16, name="wqTs")
        nc.sync.dma_start(wqT, wqT_d[:])
        wkT = consts.tile([128, KO, 256], BF16, name="wkTs")
        nc.sync.dma_start(wkT, wkT_d[:])
        wvT = consts.tile([128, KO, 256], BF16, name="wvTs")
        nc.sync.dma_start(wvT, wvT_d[:])
        woT = consts.tile([128, 2, EMBED], BF16, name="woTs")
        nc.sync.dma_start(woT, woT_d[:])
        bq = consts.tile([128, 2], F32, name="bqs")
        nc.sync.dma_start(bq, bq_d[:])
        bk = consts.tile([128, 2], F32, name="bks")
        nc.sync.dma_start(bk, bk_d[:])
        bvb = consts.tile([128, 256], F32, name="bvbs")
        nc.sync.dma_start(bvb, bv_d[:])
        maskM = consts.tile([128, 512], BF16, name="masks")
        nc.sync.dma_start(maskM, mk_d[:])

        # Persistent intermediates.
        # QT/KT: [d-part(64*eo+dl), hp, s]; Vp: [t-part, ttile, head, 64+ones]
        QT = consts.tile([128, 2, S], BF16, name="QTs")
        KT = consts.tile([128, 2, S], BF16, name="KTs")
        Vp = consts.tile([128, NTT, 4, 65], BF16, name="Vps")
        AO = consts.tile([128, 2, S], BF16, name="AOs")  # attnout^T per hp pair
        nc.vector.memset(Vp[:, :, :, 64:65], 1.0)

        # ---------------- Phase 1: Q/K/V projections ----------------
        with tc.tile_pool(name="qk_psum", bufs=1, space="PSUM") as qkp, tc.tile_pool(
            name="v_psum", bufs=2, space="PSUM"
        ) as vpp:
            for W, bias_sb, OUT, nm in ((wqT, bq, QT, "q"), (wkT, bk, KT, "k")):
                for hp in range(2):
                    psums = [
                        qkp.tile([128, 512], F32, name=f"qkps{sb}", tag=f"qkps{sb}")
                        for sb in range(NSB)
                    ]
                    for k in range(KO):
                        for sb in range(NSB):
                            nc.tensor.matmul(
                                psums[sb],
                                W[:, k, 128 * hp : 128 * (hp + 1)],
                                xT[:, k, 512 * sb : 512 * (sb + 1)],
                                start=(k == 0),
                                stop=(k == KO - 1),
                            )
                    for sb in range(NSB):
                        nc.vector.tensor_scalar_add(
                            OUT[:, hp, 512 * sb : 512 * (sb + 1)],
                            psums[sb],
                            bq[:, hp : hp + 1] if nm == "q" else bk[:, hp : hp + 1],
                        )
            for tt in range(NTT):
                vps = vpp.tile([128, 256], F32, name="vps", tag="vps")
                for k in range(KO):
                    nc.tensor.matmul(
                        vps,
                        xT[:, k, 128 * tt : 128 * (tt + 1)],
                        wvT[:, k, :],
                        start=(k == 0),
                        stop=(k == KO - 1),
                    )
                nc.vector.tensor_tensor(
                    Vp[:, tt, :, 0:64],
                    vps.rearrange("p (h d) -> p h d", h=4),
                    bvb.rearrange("p (h d) -> p h d", h=4),
                    ALU.add,
                )

        # ---------------- Phase 2: attention + out-proj ----------------
        with tc.tile_pool(name="sc_psum", bufs=3, space="PSUM") as scp, tc.tile_pool(
            name="pvy_psum", bufs=2, space="PSUM"
        ) as pvyp, tc.tile_pool(name="ptp", bufs=2) as ptp, tc.tile_pool(
            name="stg", bufs=3
        ) as stg, tc.tile_pool(name="dscr", bufs=3, space="DRAM") as dscr:
            for sb in range(NSB):
                for hp in range(2):
                    nt = 4 * sb + 4  # t-tiles (causal)
                    PT = ptp.tile([128, 2, NTT, 512], BF16, name="PT", tag="PT")
                    # scores + exp, one k-tile at a time (both heads of pair)
                    for k in range(nt):
                        r = k - 4 * sb
                        off = 128 * r if r > 0 else 0
                        sc = scp.tile([128, 2, 512], F32, name="sc", tag="sc")
                        for eo in range(2):
                            nc.tensor.matmul(
                                sc[:, eo, off:512],
                                KT[64 * eo : 64 * eo + 64, hp, 128 * k : 128 * (k + 1)],
                                QT[
                                    64 * eo : 64 * eo + 64,
                                    hp,
                                    512 * sb + off : 512 * (sb + 1),
                                ],
                                start=True,
                                stop=True,
                            )
                        nc.scalar.activation(
                            PT[:, :, k, :], sc, AF.Exp, bias=0.0, scale=SCALE
                        )
                    # causal mask on the 4 diagonal k-tiles
                    for k in range(4 * sb, nt):
                        off = 128 * (k - 4 * sb)
                        w = 512 - off
                        for eo in range(2):
                            nc.vector.tensor_tensor(
                                PT[:, eo, k, off:512],
                                PT[:, eo, k, off:512],
                                maskM[:, 0:w],
                                ALU.mult,
                            )
                    # PV: accumulate [65, 512] = V'^T @ P^T  (row 64 = denom)
                    pvs = []
                    for eo in range(2):
                        pv = pvyp.tile([128, 512], F32, name=f"pv{eo}", tag="pvy")
                        for k in range(nt):
                            r = k - 4 * sb
                            off = 128 * r if r > 0 else 0
                            nc.tensor.matmul(
                                pv[0:65, off:512],
                                Vp[:, k, 2 * hp + eo, :],
                                PT[:, eo, k, off:512],
                                start=(k == 0),
                                stop=(k == nt - 1),
                            )
                        pvs.append(pv)
                    # normalize: recip of denom row, broadcast, multiply
                    rc = stg.tile([65, 2, 512], F32, name="rc", tag="rc")
                    scr = dscr.tile([2, 512], F32, name="scr", tag="scr")
                    for eo in range(2):
                        nc.vector.reciprocal(rc[64:65, eo, :], pvs[eo][64:65, :])
                        nc.sync.dma_start(scr[eo : eo + 1, :], rc[64:65, eo, :])
                    for eo in range(2):
                        bc = stg.tile([64, 512], F32, name=f"bc{eo}", tag=f"bc{eo}")
                        nc.sync.dma_start(bc, scr[eo : eo + 1, :].to_broadcast((64, 512)))
                        if eo == 0:
                            nc.vector.tensor_tensor(
                                AO[0:64, hp, 512 * sb : 512 * (sb + 1)],
                                pvs[eo][0:64, :],
                                bc,
                                ALU.mult,
                            )
                        else:
                            aos = stg.tile([64, 512], BF16, name="aos", tag="aos")
                            nc.vector.tensor_tensor(aos, pvs[eo][0:64, :], bc, ALU.mult)
                            nc.sync.dma_start(
                                AO[64:128, hp, 512 * sb : 512 * (sb + 1)], aos
                            )
                # out-proj for this s block
                for i in range(4):
                    st = 4 * sb + i
                    for eb in range(2):
                        yt = pvyp.tile([128, 512], F32, name="yt", tag="pvy")
                        for hp in range(2):
                            nc.tensor.matmul(
                                yt,
                                AO[:, hp, 128 * st : 128 * (st + 1)],
                                woT[:, hp, 512 * eb : 512 * (eb + 1)],
                                start=(hp == 0),
                                stop=(hp == 1),
                            )
                        ysb = stg.tile([128, 512], F32, name="ysb", tag="ysb")
                        nc.vector.tensor_copy(ysb, yt)
                        nc.sync.dma_start(yv[st][:, 512 * eb : 512 * (eb + 1)], ysb)
    nc.finalize()
    return nc


# ---------------------------------------------------------------------------


def _part3(a2d, n_inner=128):
    """[D0, D1] -> [128, D0//128, D1] with partition-inner grouping of dim 0."""
    d0, d1 = a2d.shape
    return np.ascontiguousarray(
        a2d.reshape(d0 // n_inner, n_inner, d1).transpose(1, 0, 2)
    )


def prep_core_inputs(x, Wq, bq, Wk, bk, Wv, bv, Wo, b, g, S):
    bf = ml_dtypes.bfloat16
    sl = slice(256 * g, 256 * (g + 1))
    xt = _part3(np.ascontiguousarray(x[b].T)).astype(bf)  # [128, KO, S]
    wqt = _part3(np.ascontiguousarray(Wq[sl, :].T)).astype(bf)  # [128, KO, 256]
    wkt = _part3(np.ascontiguousarray(Wk[sl, :].T)).astype(bf)
    wvt = _part3(np.ascontiguousarray(Wv[sl, :].T)).astype(bf)
    wot = _part3(np.ascontiguousarray(Wo[:, sl].T)).astype(bf)  # [128, 2, 1024]
    bq2 = np.ascontiguousarray(bq[sl].reshape(2, 128).T).astype(np.float32)
    bk2 = np.ascontiguousarray(bk[sl].reshape(2, 128).T).astype(np.float32)
    bvb = np.ascontiguousarray(np.broadcast_to(bv[sl], (128, 256))).astype(np.float32)
    mask = (np.arange(512)[None, :] >= np.arange(128)[:, None]).astype(bf)
    return {
        "xT": xt,
        "wqT": wqt,
        "wkT": wkt,
        "wvT": wvt,
        "woT": wot,
        "bq2": bq2,
        "bk2": bk2,
        "bvb": bvb,
        "maskM": mask,
    }


# ---------------------------------------------------------------------------
# PJRT runner: jit once, execute n_exec times (first execution after a fresh
# NEFF load has been observed to be flaky), return the last result.


def run_spmd(nc, in_maps, n_cores, n_exec=2):
    import jax
    from jax.sharding import Mesh, PartitionSpec
    from jax.experimental.shard_map import shard_map
    from concourse import bass2jax

    bass2jax.install_neuronx_cc_hook()

    partition_name = (
        nc.partition_id_tensor.name if nc.partition_id_tensor else None
    )
    in_names, out_names, out_avals, zero_outs = [], [], [], []
    for alloc in nc.m.functions[0].allocations:
        if not isinstance(alloc, mybir.MemoryLocationSet):
            continue
        name = alloc.memorylocations[0].name
        if alloc.kind == "ExternalInput":
            if name != partition_name:
                in_names.append(name)
        elif alloc.kind == "ExternalOutput":
            shape = tuple(alloc.tensor_shape)
            dtype = mybir.dt.np(alloc.dtype)
            out_names.append(name)
            out_avals.append(jax.core.ShapedArray(shape, dtype))
            zero_outs.append(np.zeros(shape, dtype))
    n_params = len(in_names)
    n_outs = len(out_avals)
    all_in_names = list(in_names) + list(out_names)
    if partition_name is not None:
        all_in_names.append(partition_name)
    donate = tuple(range(n_params, n_params + n_outs))

    def _body(*args):
        operands = list(args)
        if partition_name is not None:
            operands.append(bass2jax.partition_id_tensor())
        outs = bass2jax._bass_exec_p.bind(
            *operands,
            out_avals=tuple(out_avals),
            in_names=tuple(all_in_names),
            out_names=tuple(out_names),
            lowering_input_output_aliases=(),
            sim_require_finite=True,
            sim_require_nnan=True,
            nc=nc,
        )
        return tuple(outs)

    devices = jax.devices()[:n_cores]
    mesh = Mesh(np.asarray(devices), ("core",))
    sharded = jax.jit(
        shard_map(
            _body,
            mesh=mesh,
            in_specs=(PartitionSpec("core"),) * (n_params + n_outs),
            out_specs=(PartitionSpec("core"),) * n_outs,
            check_rep=False,
        ),
        donate_argnums=donate,
        keep_unused=True,
    )
    per_core = [[np.asarray(m[name]) for name in in_names] for m in in_maps]
    concat_in = [
        np.concatenate([per_core[c][i] for c in range(n_cores)], axis=0)
        for i in range(n_params)
    ]
    out_arrs = None
    for _ in range(max(1, n_exec)):
        concat_zeros = [
            np.zeros((n_cores * z.shape[0], *z.shape[1:]), z.dtype) for z in zero_outs
        ]
        out_arrs = sharded(*concat_in, *concat_zeros)
    return [
        {
            name: np.asarray(out_arrs[i]).reshape(n_cores, *out_avals[i].shape)[c]
            for i, name in enumerate(out_names)
        }
        for c in range(n_cores)
    ]


_NC_CACHE = {}


def kernel(x, Wq, bq, Wk, bk, Wv, bv, Wo, bo):
    x = np.asarray(x, dtype=np.float32)
    Wq = np.asarray(Wq, dtype=np.float32)
    bq = np.asarray(bq, dtype=np.float32)
    Wk = np.asarray(Wk, dtype=np.float32)
    bk = np.asarray(bk, dtype=np.float32)
    Wv = np.asarray(Wv, dtype=np.float32)
    bv = np.asarray(bv, dtype=np.float32)
    Wo = np.asarray(Wo, dtype=np.float32)
    bo = np.asarray(bo, dtype=np.float32)

    B, S, E = x.shape
    assert E == EMBED
    if S not in _NC_CACHE:
        _NC_CACHE[S] = build_nc(S)
    nc = _NC_CACHE[S]

    in_maps = [
        prep_core_inputs(x, Wq, bq, Wk, bk, Wv, bv, Wo, c // 4, c % 4, S)
        for c in range(N_CORES)
    ]
    results = run_spmd(nc, in_maps, N_CORES)
    out = np.empty((B, S, E), dtype=np.float32)
    for b in range(B):
        acc = results[4 * b]["y"].astype(np.float32).copy()
        for g in range(1, 4):
            acc += results[4 * b + g]["y"]
        out[b] = acc + bo[None, :]
    return out


# revision 8
# speedup vs baseline: 1.1511x; 1.1511x over previous
"""Multi-head causal attention (B=2, S=2048, E=1024, H=16) on 8 NeuronCores.

Sharding: one core per (batch, head-group-of-4). Each core:
  - computes Q/K/V projections for its 256 embed dims (bf16 matmuls, fp32 psum)
  - causal attention for its 4 heads (scores transposed [t,s]; softmax
    denominators via a ones-row appended to V; exp on ScalarE from PSUM)
  - partial output projection y_part = attnout @ Wo_cols^T
Host sums the 4 partials per batch and adds bo.
"""

import numpy as np
import ml_dtypes
from contextlib import ExitStack

import concourse.bacc as bacc
import concourse.mybir as mybir
import concourse.tile as tile

BF16 = mybir.dt.bfloat16
F32 = mybir.dt.float32
AF = mybir.ActivationFunctionType
ALU = mybir.AluOpType

EMBED = 1024
KO = EMBED // 128  # 8 contraction tiles over embed
N_CORES = 8
SCALE = 0.125  # 1/sqrt(64)


def build_nc(S):
    """Emit the per-core kernel. Identical program on all cores (SPMD)."""
    NSB = S // 512  # 512-wide s blocks
    NTT = S // 128  # 128-wide s/t tiles

    nc = bacc.Bacc("TRN2", target_bir_lowering=False)

    xT_d = nc.dram_tensor("xT", [128, KO, S], BF16, kind="ExternalInput")
    wqT_d = nc.dram_tensor("wqT", [128, KO, 256], BF16, kind="ExternalInput")
    wkT_d = nc.dram_tensor("wkT", [128, KO, 256], BF16, kind="ExternalInput")
    wvT_d = nc.dram_tensor("wvT", [128, KO, 256], BF16, kind="ExternalInput")
    woT_d = nc.dram_tensor("woT", [128, 2, EMBED], BF16, kind="ExternalInput")
    bq_d = nc.dram_tensor("bq2", [128, 2], F32, kind="ExternalInput")
    bk_d = nc.dram_tensor("bk2", [128, 2], F32, kind="ExternalInput")
    bv_d = nc.dram_tensor("bvb", [128, 256], F32, kind="ExternalInput")
    mk_d = nc.dram_tensor("maskM", [128, 512], BF16, kind="ExternalInput")
    y_d = nc.dram_tensor("y", [S, EMBED], F32, kind="ExternalOutput")
    yv = y_d[:].rearrange("(so p) e -> so p e", p=128)

    with tile.TileContext(nc) as tc, ExitStack() as ctx:
        consts = ctx.enter_context(tc.tile_pool(name="consts", bufs=1))

        # weights first so the first projection matmuls can start early,
        # then x one contraction tile at a time in consumption order
        wqT = consts.tile([128, KO, 256], BF16, name="wqTs")
        nc.sync.dma_start(wqT, wqT_d[:])
        bq = consts.tile([128, 2], F32, name="bqs")
        nc.sync.dma_start(bq, bq_d[:])
        xT = consts.tile([128, KO, S], BF16, name="xTs")
        for k in range(KO):
            nc.sync.dma_start(xT[:, k, :], xT_d[:, k, :])
        wkT = consts.tile([128, KO, 256], BF16, name="wkTs")
        nc.sync.dma_start(wkT, wkT_d[:])
        bk = consts.tile([128, 2], F32, name="bks")
        nc.sync.dma_start(bk, bk_d[:])
        wvT = consts.tile([128, KO, 256], BF16, name="wvTs")
        nc.sync.dma_start(wvT, wvT_d[:])
        bvb = consts.tile([128, 256], F32, name="bvbs")
        nc.sync.dma_start(bvb, bv_d[:])
        maskM = consts.tile([128, 512], BF16, name="masks")
        nc.sync.dma_start(maskM, mk_d[:])
        woT = consts.tile([128, 2, EMBED], BF16, name="woTs")
        nc.sync.dma_start(woT, woT_d[:])

        # Persistent intermediates.
        # QT/KT: [d-part(64*eo+dl), hp, s]; Vp: [t-part, ttile, head, 64+ones]
        QT = consts.tile([128, 2, S], BF16, name="QTs")
        KT = consts.tile([128, 2, S], BF16, name="KTs")
        Vp = consts.tile([128, NTT, 4, 65], BF16, name="Vps")
        AO = consts.tile([128, 2, S], BF16, name="AOs")  # attnout^T per hp pair
        nc.vector.memset(Vp[:, :, :, 64:65], 1.0)

        # ---------------- Phase 1: Q/K/V projections ----------------
        with tc.tile_pool(name="qk_psum", bufs=1, space="PSUM") as qkp, tc.tile_pool(
            name="v_psum", bufs=2, space="PSUM"
        ) as vpp:
            for W, bias_sb, OUT, nm in ((wqT, bq, QT, "q"), (wkT, bk, KT, "k")):
                for hp in range(2):
                    psums = [
                        qkp.tile([128, 512], F32, name=f"qkps{sb}", tag=f"qkps{sb}")
                        for sb in range(NSB)
                    ]
                    for k in range(KO):
                        for sb in range(NSB):
                            nc.tensor.matmul(
                                psums[sb],
                                W[:, k, 128 * hp : 128 * (hp + 1)],
                                xT[:, k, 512 * sb : 512 * (sb + 1)],
                                start=(k == 0),
                                stop=(k == KO - 1),
                            )
                    for sb in range(NSB):
                        nc.vector.tensor_scalar_add(
                            OUT[:, hp, 512 * sb : 512 * (sb + 1)],
                            psums[sb],
                            bq[:, hp : hp + 1] if nm == "q" else bk[:, hp : hp + 1],
                        )
            for tt in range(NTT):
                vps = vpp.tile([128, 256], F32, name="vps", tag="vps")
                for k in range(KO):
                    nc.tensor.matmul(
                        vps,
                        xT[:, k, 128 * tt : 128 * (tt + 1)],
                        wvT[:, k, :],
                        start=(k == 0),
                        stop=(k == KO - 1),
                    )
                nc.vector.tensor_tensor(
                    Vp[:, tt, :, 0:64],
                    vps.rearrange("p (h d) -> p h d", h=4),
                    bvb.rearrange("p (h d) -> p h d", h=4),
                    ALU.add,
                )

        # ---------------- Phase 2: attention + out-proj ----------------
        with tc.tile_pool(name="sc_psum", bufs=3, space="PSUM") as scp, tc.tile_pool(
            name="pvy_psum", bufs=2, space="PSUM"
        ) as pvyp, tc.tile_pool(name="ptp", bufs=2) as ptp, tc.tile_pool(
            name="stg", bufs=3
        ) as stg, tc.tile_pool(name="dscr", bufs=3, space="DRAM") as dscr:

            def emit_yproj(sb):
                for i in range(4):
                    st = 4 * sb + i
                    for eb in range(2):
                        yt = pvyp.tile([128, 512], F32, name="yt", tag="pvy")
                        for hp in range(2):
                            nc.tensor.matmul(
                                yt,
                                AO[:, hp, 128 * st : 128 * (st + 1)],
                                woT[:, hp, 512 * eb : 512 * (eb + 1)],
                                start=(hp == 0),
                                stop=(hp == 1),
                            )
                        ysb = stg.tile([128, 512], F32, name="ysb", tag="ysb")
                        nc.vector.tensor_copy(ysb, yt)
                        nc.sync.dma_start(yv[st][:, 512 * eb : 512 * (eb + 1)], ysb)

            LAG = 2
            for sb in range(NSB):
                for hp in range(2):
                    nt = 4 * sb + 4  # t-tiles (causal)
                    PT = ptp.tile([128, 2, NTT, 512], BF16, name="PT", tag="PT")
                    pvs = [
                        pvyp.tile([128, 512], F32, name=f"pv{eo}", tag="pvy")
                        for eo in range(2)
                    ]

                    def emit_pv(k):
                        r = k - 4 * sb
                        off = 128 * r if r > 0 else 0
                        for eo in range(2):
                            nc.tensor.matmul(
                                pvs[eo][0:65, off:512],
                                Vp[:, k, 2 * hp + eo, :],
                                PT[:, eo, k, off:512],
                                start=(k == 0),
                                stop=(k == nt - 1),
                            )

                    # scores + exp + (lagged) PV, interleaved per k-tile
                    for k in range(nt):
                        r = k - 4 * sb
                        off = 128 * r if r > 0 else 0
                        sc = scp.tile([128, 2, 512], F32, name="sc", tag="sc")
                        for eo in range(2):
                            nc.tensor.matmul(
                                sc[:, eo, off:512],
                                KT[64 * eo : 64 * eo + 64, hp, 128 * k : 128 * (k + 1)],
                                QT[
                                    64 * eo : 64 * eo + 64,
                                    hp,
                                    512 * sb + off : 512 * (sb + 1),
                                ],
                                start=True,
                                stop=True,
                            )
                        nc.scalar.activation(
                            PT[:, :, k, off:512],
                            sc[:, :, off:512],
                            AF.Exp,
                            bias=0.0,
                            scale=SCALE,
                        )
                        if r >= 0:  # diagonal: causal mask
                            for eo in range(2):
                                nc.vector.tensor_tensor(
                                    PT[:, eo, k, off:512],
                                    PT[:, eo, k, off:512],
                                    maskM[:, 0 : 512 - off],
                                    ALU.mult,
                                )
                        if k >= LAG:
                            emit_pv(k - LAG)
                    for k in range(max(0, nt - LAG), nt):
                        emit_pv(k)
                    # normalize: denom row -> SBUF -> DRAM -> broadcast [64,512],
                    # approx-reciprocal at base partition 0, multiply
                    rc = stg.tile([65, 2, 512], F32, name="rc", tag="rc")
                    scr = dscr.tile([2, 512], F32, name="scr", tag="scr")
                    for eo in range(2):
                        nc.vector.tensor_copy(rc[64:65, eo, :], pvs[eo][64:65, :])
                        nc.sync.dma_start(scr[eo : eo + 1, :], rc[64:65, eo, :])
                    for eo in range(2):
                        bc = stg.tile([64, 512], F32, name=f"bc{eo}", tag=f"bc{eo}")
                        nc.sync.dma_start(bc, scr[eo : eo + 1, :].to_broadcast((64, 512)))
                        rcp = stg.tile([64, 512], F32, name=f"rcp{eo}", tag=f"rcp{eo}")
                        nc.vector.reciprocal_approx_fast(rcp, bc)
                        if eo == 0:
                            nc.vector.tensor_tensor(
                                AO[0:64, hp, 512 * sb : 512 * (sb + 1)],
                                pvs[eo][0:64, :],
                                rcp,
                                ALU.mult,
                            )
                        else:
                            aos = stg.tile([64, 512], BF16, name="aos", tag="aos")
                            nc.vector.tensor_tensor(aos, pvs[eo][0:64, :], rcp, ALU.mult)
                            nc.sync.dma_start(
                                AO[64:128, hp, 512 * sb : 512 * (sb + 1)], aos
                            )
                    # out-proj of the previous s block between the two head
                    # pairs: hides the normalize chain latency
                    if hp == 0 and sb >= 1:
                        emit_yproj(sb - 1)
            emit_yproj(NSB - 1)
    nc.finalize()
    return nc


# ---------------------------------------------------------------------------


def _part3(a2d, n_inner=128):
    """[D0, D1] -> [128, D0//128, D1] with partition-inner grouping of dim 0."""
    d0, d1 = a2d.shape
    return np.ascontiguousarray(
        a2d.reshape(d0 // n_inner, n_inner, d1).transpose(1, 0, 2)
    )


def prep_core_inputs(x, Wq, bq, Wk, bk, Wv, bv, Wo, b, g, S):
    bf = ml_dtypes.bfloat16
    sl = slice(256 * g, 256 * (g + 1))
    xt = _part3(np.ascontiguousarray(x[b].T)).astype(bf)  # [128, KO, S]
    wqt = _part3(np.ascontiguousarray(Wq[sl, :].T)).astype(bf)  # [128, KO, 256]
    wkt = _part3(np.ascontiguousarray(Wk[sl, :].T)).astype(bf)
    wvt = _part3(np.ascontiguousarray(Wv[sl, :].T)).astype(bf)
    wot = _part3(np.ascontiguousarray(Wo[:, sl].T)).astype(bf)  # [128, 2, 1024]
    bq2 = np.ascontiguousarray(bq[sl].reshape(2, 128).T).astype(np.float32)
    bk2 = np.ascontiguousarray(bk[sl].reshape(2, 128).T).astype(np.float32)
    bvb = np.ascontiguousarray(np.broadcast_to(bv[sl], (128, 256))).astype(np.float32)
    mask = (np.arange(512)[None, :] >= np.arange(128)[:, None]).astype(bf)
    return {
        "xT": xt,
        "wqT": wqt,
        "wkT": wkt,
        "wvT": wvt,
        "woT": wot,
        "bq2": bq2,
        "bk2": bk2,
        "bvb": bvb,
        "maskM": mask,
    }


# ---------------------------------------------------------------------------
# PJRT runner: jit once, execute n_exec times (first execution after a fresh
# NEFF load has been observed to be flaky), return the last result.


def run_spmd(nc, in_maps, n_cores, n_exec=2):
    import jax
    from jax.sharding import Mesh, PartitionSpec
    from jax.experimental.shard_map import shard_map
    from concourse import bass2jax

    bass2jax.install_neuronx_cc_hook()

    partition_name = (
        nc.partition_id_tensor.name if nc.partition_id_tensor else None
    )
    in_names, out_names, out_avals, zero_outs = [], [], [], []
    for alloc in nc.m.functions[0].allocations:
        if not isinstance(alloc, mybir.MemoryLocationSet):
            continue
        name = alloc.memorylocations[0].name
        if alloc.kind == "ExternalInput":
            if name != partition_name:
                in_names.append(name)
        elif alloc.kind == "ExternalOutput":
            shape = tuple(alloc.tensor_shape)
            dtype = mybir.dt.np(alloc.dtype)
            out_names.append(name)
            out_avals.append(jax.core.ShapedArray(shape, dtype))
            zero_outs.append(np.zeros(shape, dtype))
    n_params = len(in_names)
    n_outs = len(out_avals)
    all_in_names = list(in_names) + list(out_names)
    if partition_name is not None:
        all_in_names.append(partition_name)
    donate = tuple(range(n_params, n_params + n_outs))

    def _body(*args):
        operands = list(args)
        if partition_name is not None:
            operands.append(bass2jax.partition_id_tensor())
        outs = bass2jax._bass_exec_p.bind(
            *operands,
            out_avals=tuple(out_avals),
            in_names=tuple(all_in_names),
            out_names=tuple(out_names),
            lowering_input_output_aliases=(),
            sim_require_finite=True,
            sim_require_nnan=True,
            nc=nc,
        )
        return tuple(outs)

    devices = jax.devices()[:n_cores]
    mesh = Mesh(np.asarray(devices), ("core",))
    sharded = jax.jit(
        shard_map(
            _body,
            mesh=mesh,
            in_specs=(PartitionSpec("core"),) * (n_params + n_outs),
            out_specs=(PartitionSpec("core"),) * n_outs,
            check_rep=False,
        ),
        donate_argnums=donate,
        keep_unused=True,
    )
    per_core = [[np.asarray(m[name]) for name in in_names] for m in in_maps]
    concat_in = [
        np.concatenate([per_core[c][i] for c in range(n_cores)], axis=0)
        for i in range(n_params)
    ]
    out_arrs = None
    for _ in range(max(1, n_exec)):
        concat_zeros = [
            np.zeros((n_cores * z.shape[0], *z.shape[1:]), z.dtype) for z in zero_outs
        ]
        out_arrs = sharded(*concat_in, *concat_zeros)
    return [
        {
            name: np.asarray(out_arrs[i]).reshape(n_cores, *out_avals[i].shape)[c]
            for i, name in enumerate(out_names)
        }
        for c in range(n_cores)
    ]


_NC_CACHE = {}


def kernel(x, Wq, bq, Wk, bk, Wv, bv, Wo, bo):
    x = np.asarray(x, dtype=np.float32)
    Wq = np.asarray(Wq, dtype=np.float32)
    bq = np.asarray(bq, dtype=np.float32)
    Wk = np.asarray(Wk, dtype=np.float32)
    bk = np.asarray(bk, dtype=np.float32)
    Wv = np.asarray(Wv, dtype=np.float32)
    bv = np.asarray(bv, dtype=np.float32)
    Wo = np.asarray(Wo, dtype=np.float32)
    bo = np.asarray(bo, dtype=np.float32)

    B, S, E = x.shape
    assert E == EMBED
    if S not in _NC_CACHE:
        _NC_CACHE[S] = build_nc(S)
    nc = _NC_CACHE[S]

    in_maps = [
        prep_core_inputs(x, Wq, bq, Wk, bk, Wv, bv, Wo, c // 4, c % 4, S)
        for c in range(N_CORES)
    ]
    results = run_spmd(nc, in_maps, N_CORES)
    out = np.empty((B, S, E), dtype=np.float32)
    for b in range(B):
        acc = results[4 * b]["y"].astype(np.float32).copy()
        for g in range(1, 4):
            acc += results[4 * b + g]["y"]
        out[b] = acc + bo[None, :]
    return out


# revision 10
# speedup vs baseline: 1.2601x; 1.0947x over previous
"""Multi-head causal attention (B=2, S=2048, E=1024, H=16) on 8 NeuronCores.

Sharding: one core per (batch, head-group-of-4). Each core:
  - computes Q/K/V projections for its 256 embed dims (bf16 matmuls, fp32 psum)
  - causal attention for its 4 heads (scores transposed [t,s]; softmax
    denominators via a ones-row appended to V; exp on ScalarE from PSUM)
  - partial output projection y_part = attnout @ Wo_cols^T
Host sums the 4 partials per batch and adds bo.
"""

import numpy as np
import ml_dtypes
from contextlib import ExitStack

import concourse.bacc as bacc
import concourse.mybir as mybir
import concourse.tile as tile

BF16 = mybir.dt.bfloat16
F32 = mybir.dt.float32
AF = mybir.ActivationFunctionType
ALU = mybir.AluOpType

EMBED = 1024
KO = EMBED // 128  # 8 contraction tiles over embed
N_CORES = 8
SCALE = 0.125  # 1/sqrt(64)


def build_nc(S):
    """Emit the per-core kernel. Identical program on all cores (SPMD)."""
    NSB = S // 512  # 512-wide s blocks
    NTT = S // 128  # 128-wide s/t tiles

    nc = bacc.Bacc("TRN2", target_bir_lowering=False)

    xT_d = nc.dram_tensor("xT", [128, KO, S], BF16, kind="ExternalInput")
    wqT_d = nc.dram_tensor("wqT", [128, KO, 256], BF16, kind="ExternalInput")
    wkT_d = nc.dram_tensor("wkT", [128, KO, 256], BF16, kind="ExternalInput")
    wvT_d = nc.dram_tensor("wvT", [128, KO, 256], BF16, kind="ExternalInput")
    woT_d = nc.dram_tensor("woT", [128, 2, EMBED], BF16, kind="ExternalInput")
    bq_d = nc.dram_tensor("bq2", [128, 2], F32, kind="ExternalInput")
    bk_d = nc.dram_tensor("bk2", [128, 2], F32, kind="ExternalInput")
    bv_d = nc.dram_tensor("bvb", [128, 256], F32, kind="ExternalInput")
    mk_d = nc.dram_tensor("maskM", [128, 512], BF16, kind="ExternalInput")
    y_d = nc.dram_tensor("y", [S, EMBED], F32, kind="ExternalOutput")
    yv = y_d[:].rearrange("(so p) e -> so p e", p=128)

    with tile.TileContext(nc) as tc, ExitStack() as ctx:
        consts = ctx.enter_context(tc.tile_pool(name="consts", bufs=1))

        # weights first so the first projection matmuls can start early,
        # then x one contraction tile at a time in consumption order
        wqT = consts.tile([128, KO, 256], BF16, name="wqTs")
        nc.sync.dma_start(wqT, wqT_d[:])
        bq = consts.tile([128, 2], F32, name="bqs")
        nc.sync.dma_start(bq, bq_d[:])
        xT = consts.tile([128, KO, S], BF16, name="xTs")
        for k in range(KO):
            nc.sync.dma_start(xT[:, k, :], xT_d[:, k, :])
        wkT = consts.tile([128, KO, 256], BF16, name="wkTs")
        nc.sync.dma_start(wkT, wkT_d[:])
        bk = consts.tile([128, 2], F32, name="bks")
        nc.sync.dma_start(bk, bk_d[:])
        wvT = consts.tile([128, KO, 256], BF16, name="wvTs")
        nc.sync.dma_start(wvT, wvT_d[:])
        bvb = consts.tile([128, 256], F32, name="bvbs")
        nc.sync.dma_start(bvb, bv_d[:])
        maskM = consts.tile([128, 512], BF16, name="masks")
        nc.sync.dma_start(maskM, mk_d[:])
        woT = consts.tile([128, 2, EMBED], BF16, name="woTs")
        nc.sync.dma_start(woT, woT_d[:])

        # Persistent intermediates.
        # QT/KT: [d-part(64*eo+dl), hp, s]; Vp: [t-part, ttile, head, 64+ones]
        QT = consts.tile([128, 2, S], BF16, name="QTs")
        KT = consts.tile([128, 2, S], BF16, name="KTs")
        Vp = consts.tile([128, NTT, 4, 65], BF16, name="Vps")
        AO = consts.tile([128, 2, S], BF16, name="AOs")  # attnout^T per hp pair
        nc.vector.memset(Vp[:, :, :, 64:65], 1.0)

        # Single software-pipelined phase: Q/K/V projection psum-groups and the
        # out-projection are interleaved as PE filler inside the (ScalarE-paced)
        # attention k-loops, keeping the PE dense and HAM-warm.
        with tc.tile_pool(name="proj_psum", bufs=2, space="PSUM") as prp, tc.tile_pool(
            name="sc_psum", bufs=2, space="PSUM"
        ) as scp, tc.tile_pool(name="pvy_psum", bufs=2, space="PSUM") as pvyp, tc.tile_pool(
            name="ptp", bufs=2
        ) as ptp, tc.tile_pool(name="stg", bufs=3) as stg, tc.tile_pool(
            name="dscr", bufs=3, space="DRAM"
        ) as dscr:

            def emit_q_group(hp, sb):
                ps = prp.tile([128, 512], F32, name="prps", tag="prps")
                for k in range(KO):
                    nc.tensor.matmul(
                        ps,
                        wqT[:, k, 128 * hp : 128 * (hp + 1)],
                        xT[:, k, 512 * sb : 512 * (sb + 1)],
                        start=(k == 0),
                        stop=(k == KO - 1),
                    )
                nc.vector.tensor_scalar_add(
                    QT[:, hp, 512 * sb : 512 * (sb + 1)], ps, bq[:, hp : hp + 1]
                )

            def emit_k_group(hp, tb):
                ps = prp.tile([128, 512], F32, name="prps", tag="prps")
                for k in range(KO):
                    nc.tensor.matmul(
                        ps,
                        wkT[:, k, 128 * hp : 128 * (hp + 1)],
                        xT[:, k, 512 * tb : 512 * (tb + 1)],
                        start=(k == 0),
                        stop=(k == KO - 1),
                    )
                nc.vector.tensor_scalar_add(
                    KT[:, hp, 512 * tb : 512 * (tb + 1)], ps, bk[:, hp : hp + 1]
                )

            def emit_v_group(tt):
                ps = prp.tile([128, 512], F32, name="prps", tag="prps")
                for k in range(KO):
                    nc.tensor.matmul(
                        ps[:, 0:256],
                        xT[:, k, 128 * tt : 128 * (tt + 1)],
                        wvT[:, k, :],
                        start=(k == 0),
                        stop=(k == KO - 1),
                    )
                nc.vector.tensor_tensor(
                    Vp[:, tt, :, 0:64],
                    ps[:, 0:256].rearrange("p (h d) -> p h d", h=4),
                    bvb.rearrange("p (h d) -> p h d", h=4),
                    ALU.add,
                )

            def emit_y_group(st, eb):
                yt = prp.tile([128, 512], F32, name="yt", tag="prps")
                for hp in range(2):
                    nc.tensor.matmul(
                        yt,
                        AO[:, hp, 128 * st : 128 * (st + 1)],
                        woT[:, hp, 512 * eb : 512 * (eb + 1)],
                        start=(hp == 0),
                        stop=(hp == 1),
                    )
                ysb = stg.tile([128, 512], F32, name="ysb", tag="ysb")
                nc.vector.tensor_copy(ysb, yt)
                nc.sync.dma_start(yv[st][:, 512 * eb : 512 * (eb + 1)], ysb)

            # Prologue projections: everything attention block sb=0 needs.
            for hp in range(2):
                emit_q_group(hp, 0)
                emit_k_group(hp, 0)
            for tt in range(4):
                emit_v_group(tt)

            LAG = 2
            for sb in range(NSB):
                # filler emitted during this sb's attention k-steps:
                # next block's projections + previous block's out-proj
                filler = []
                if sb + 1 < NSB:
                    for hp in range(2):
                        filler.append(lambda hp=hp: emit_q_group(hp, sb + 1))
                        filler.append(lambda hp=hp: emit_k_group(hp, sb + 1))
                    for tt in range(4 * (sb + 1), 4 * (sb + 2)):
                        filler.append(lambda tt=tt: emit_v_group(tt))
                if sb >= 1:
                    for i in range(4):
                        for eb in range(2):
                            filler.append(
                                lambda st=4 * (sb - 1) + i, eb=eb: emit_y_group(st, eb)
                            )
                fi = iter(filler)

                for hp in range(2):
                    nt = 4 * sb + 4  # t-tiles (causal)
                    PT = ptp.tile([128, 2, NTT, 512], BF16, name="PT", tag="PT")
                    pvs = [
                        pvyp.tile([128, 512], F32, name=f"pv{eo}", tag="pvy")
                        for eo in range(2)
                    ]

                    def emit_pv(k, hp=hp, pvs=pvs, PT=PT, nt=nt):
                        r = k - 4 * sb
                        off = 128 * r if r > 0 else 0
                        for eo in range(2):
                            nc.tensor.matmul(
                                pvs[eo][0:65, off:512],
                                Vp[:, k, 2 * hp + eo, :],
                                PT[:, eo, k, off:512],
                                start=(k == 0),
                                stop=(k == nt - 1),
                            )

                    # scores + exp + (lagged) PV, interleaved per k-tile
                    for k in range(nt):
                        r = k - 4 * sb
                        off = 128 * r if r > 0 else 0
                        sc = scp.tile([128, 2, 512], F32, name="sc", tag="sc")
                        for eo in range(2):
                            nc.tensor.matmul(
                                sc[:, eo, off:512],
                                KT[64 * eo : 64 * eo + 64, hp, 128 * k : 128 * (k + 1)],
                                QT[
                                    64 * eo : 64 * eo + 64,
                                    hp,
                                    512 * sb + off : 512 * (sb + 1),
                                ],
                                start=True,
                                stop=True,
                            )
                        nc.scalar.activation(
                            PT[:, :, k, off:512],
                            sc[:, :, off:512],
                            AF.Exp,
                            bias=0.0,
                            scale=SCALE,
                        )
                        if r >= 0:  # diagonal: causal mask
                            for eo in range(2):
                                nc.vector.tensor_tensor(
                                    PT[:, eo, k, off:512],
                                    PT[:, eo, k, off:512],
                                    maskM[:, 0 : 512 - off],
                                    ALU.mult,
                                )
                        if k >= LAG:
                            emit_pv(k - LAG)
                        nxt = next(fi, None)
                        if nxt is not None:
                            nxt()
                    for k in range(max(0, nt - LAG), nt):
                        emit_pv(k)
                    # normalize: denom row -> SBUF -> DRAM -> broadcast [64,512],
                    # approx-reciprocal at base partition 0, multiply
                    rc = stg.tile([65, 2, 512], F32, name="rc", tag="rc")
                    scr = dscr.tile([2, 512], F32, name="scr", tag="scr")
                    for eo in range(2):
                        nc.vector.tensor_copy(rc[64:65, eo, :], pvs[eo][64:65, :])
                        nc.sync.dma_start(scr[eo : eo + 1, :], rc[64:65, eo, :])
                    for eo in range(2):
                        bc = stg.tile([64, 512], F32, name=f"bc{eo}", tag=f"bc{eo}")
                        nc.sync.dma_start(bc, scr[eo : eo + 1, :].to_broadcast((64, 512)))
                        rcp = stg.tile([64, 512], F32, name=f"rcp{eo}", tag=f"rcp{eo}")
                        nc.vector.reciprocal_approx_fast(rcp, bc)
                        if eo == 0:
                            nc.vector.tensor_tensor(
                                AO[0:64, hp, 512 * sb : 512 * (sb + 1)],
                                pvs[eo][0:64, :],
                                rcp,
                                ALU.mult,
                            )
                        else:
                            aos = stg.tile([64, 512], BF16, name="aos", tag="aos")
                            nc.vector.tensor_tensor(aos, pvs[eo][0:64, :], rcp, ALU.mult)
                            nc.sync.dma_start(
                                AO[64:128, hp, 512 * sb : 512 * (sb + 1)], aos
                            )
                # drain any remaining filler for this sb
                for nxt in fi:
                    nxt()
            for i in range(4):
                for eb in range(2):
                    emit_y_group(4 * (NSB - 1) + i, eb)
    nc.finalize()
    return nc


# ---------------------------------------------------------------------------


def _part3(a2d, n_inner=128):
    """[D0, D1] -> [128, D0//128, D1] with partition-inner grouping of dim 0."""
    d0, d1 = a2d.shape
    return np.ascontiguousarray(
        a2d.reshape(d0 // n_inner, n_inner, d1).transpose(1, 0, 2)
    )


def prep_core_inputs(x, Wq, bq, Wk, bk, Wv, bv, Wo, b, g, S):
    bf = ml_dtypes.bfloat16
    sl = slice(256 * g, 256 * (g + 1))
    xt = _part3(np.ascontiguousarray(x[b].T)).astype(bf)  # [128, KO, S]
    wqt = _part3(np.ascontiguousarray(Wq[sl, :].T)).astype(bf)  # [128, KO, 256]
    wkt = _part3(np.ascontiguousarray(Wk[sl, :].T)).astype(bf)
    wvt = _part3(np.ascontiguousarray(Wv[sl, :].T)).astype(bf)
    wot = _part3(np.ascontiguousarray(Wo[:, sl].T)).astype(bf)  # [128, 2, 1024]
    bq2 = np.ascontiguousarray(bq[sl].reshape(2, 128).T).astype(np.float32)
    bk2 = np.ascontiguousarray(bk[sl].reshape(2, 128).T).astype(np.float32)
    bvb = np.ascontiguousarray(np.broadcast_to(bv[sl], (128, 256))).astype(np.float32)
    mask = (np.arange(512)[None, :] >= np.arange(128)[:, None]).astype(bf)
    return {
        "xT": xt,
        "wqT": wqt,
        "wkT": wkt,
        "wvT": wvt,
        "woT": wot,
        "bq2": bq2,
        "bk2": bk2,
        "bvb": bvb,
        "maskM": mask,
    }


# ---------------------------------------------------------------------------
# PJRT runner: jit once, execute n_exec times (first execution after a fresh
# NEFF load has been observed to be flaky), return the last result.


def run_spmd(nc, in_maps, n_cores, n_exec=2):
    import jax
    from jax.sharding import Mesh, PartitionSpec
    from jax.experimental.shard_map import shard_map
    from concourse import bass2jax

    bass2jax.install_neuronx_cc_hook()

    partition_name = (
        nc.partition_id_tensor.name if nc.partition_id_tensor else None
    )
    in_names, out_names, out_avals, zero_outs = [], [], [], []
    for alloc in nc.m.functions[0].allocations:
        if not isinstance(alloc, mybir.MemoryLocationSet):
            continue
        name = alloc.memorylocations[0].name
        if alloc.kind == "ExternalInput":
            if name != partition_name:
                in_names.append(name)
        elif alloc.kind == "ExternalOutput":
            shape = tuple(alloc.tensor_shape)
            dtype = mybir.dt.np(alloc.dtype)
            out_names.append(name)
            out_avals.append(jax.core.ShapedArray(shape, dtype))
            zero_outs.append(np.zeros(shape, dtype))
    n_params = len(in_names)
    n_outs = len(out_avals)
    all_in_names = list(in_names) + list(out_names)
    if partition_name is not None:
        all_in_names.append(partition_name)
    donate = tuple(range(n_params, n_params + n_outs))

    def _body(*args):
        operands = list(args)
        if partition_name is not None:
            operands.append(bass2jax.partition_id_tensor())
        outs = bass2jax._bass_exec_p.bind(
            *operands,
            out_avals=tuple(out_avals),
            in_names=tuple(all_in_names),
            out_names=tuple(out_names),
            lowering_input_output_aliases=(),
            sim_require_finite=True,
            sim_require_nnan=True,
            nc=nc,
        )
        return tuple(outs)

    devices = jax.devices()[:n_cores]
    mesh = Mesh(np.asarray(devices), ("core",))
    sharded = jax.jit(
        shard_map(
            _body,
            mesh=mesh,
            in_specs=(PartitionSpec("core"),) * (n_params + n_outs),
            out_specs=(PartitionSpec("core"),) * n_outs,
            check_rep=False,
        ),
        donate_argnums=donate,
        keep_unused=True,
    )
    per_core = [[np.asarray(m[name]) for name in in_names] for m in in_maps]
    concat_in = [
        np.concatenate([per_core[c][i] for c in range(n_cores)], axis=0)
        for i in range(n_params)
    ]
    out_arrs = None
    for _ in range(max(1, n_exec)):
        concat_zeros = [
            np.zeros((n_cores * z.shape[0], *z.shape[1:]), z.dtype) for z in zero_outs
        ]
        out_arrs = sharded(*concat_in, *concat_zeros)
    return [
        {
            name: np.asarray(out_arrs[i]).reshape(n_cores, *out_avals[i].shape)[c]
            for i, name in enumerate(out_names)
        }
        for c in range(n_cores)
    ]


_NC_CACHE = {}


def kernel(x, Wq, bq, Wk, bk, Wv, bv, Wo, bo):
    x = np.asarray(x, dtype=np.float32)
    Wq = np.asarray(Wq, dtype=np.float32)
    bq = np.asarray(bq, dtype=np.float32)
    Wk = np.asarray(Wk, dtype=np.float32)
    bk = np.asarray(bk, dtype=np.float32)
    Wv = np.asarray(Wv, dtype=np.float32)
    bv = np.asarray(bv, dtype=np.float32)
    Wo = np.asarray(Wo, dtype=np.float32)
    bo = np.asarray(bo, dtype=np.float32)

    B, S, E = x.shape
    assert E == EMBED
    if S not in _NC_CACHE:
        _NC_CACHE[S] = build_nc(S)
    nc = _NC_CACHE[S]

    in_maps = [
        prep_core_inputs(x, Wq, bq, Wk, bk, Wv, bv, Wo, c // 4, c % 4, S)
        for c in range(N_CORES)
    ]
    results = run_spmd(nc, in_maps, N_CORES)
    out = np.empty((B, S, E), dtype=np.float32)
    for b in range(B):
        acc = results[4 * b]["y"].astype(np.float32).copy()
        for g in range(1, 4):
            acc += results[4 * b + g]["y"]
        out[b] = acc + bo[None, :]
    return out


# revision 14
# speedup vs baseline: 1.3129x; 1.0419x over previous
"""Multi-head causal attention (B=2, S=2048, E=1024, H=16) on 8 NeuronCores.

Sharding: one core per (batch, head-group-of-4). Each core:
  - computes Q/K/V projections for its 256 embed dims (bf16 matmuls, fp32 psum)
  - causal attention for its 4 heads (scores transposed [t,s]; softmax
    denominators via a ones-row appended to V; exp on ScalarE from PSUM)
  - partial output projection y_part = attnout @ Wo_cols^T
Host sums the 4 partials per batch and adds bo.
"""

import numpy as np
import ml_dtypes
from contextlib import ExitStack

import concourse.bacc as bacc
import concourse.mybir as mybir
import concourse.tile as tile

BF16 = mybir.dt.bfloat16
F32 = mybir.dt.float32
AF = mybir.ActivationFunctionType
ALU = mybir.AluOpType

EMBED = 1024
KO = EMBED // 128  # 8 contraction tiles over embed
N_CORES = 8
SCALE = 0.125  # 1/sqrt(64)


def build_nc(S):
    """Emit the per-core kernel. Identical program on all cores (SPMD)."""
    NSB = S // 512  # 512-wide s blocks
    NTT = S // 128  # 128-wide s/t tiles

    nc = bacc.Bacc("TRN2", target_bir_lowering=False)

    xT_d = nc.dram_tensor("xT", [128, KO, S], BF16, kind="ExternalInput")
    wqT_d = nc.dram_tensor("wqT", [128, KO, 256], BF16, kind="ExternalInput")
    wkT_d = nc.dram_tensor("wkT", [128, KO, 256], BF16, kind="ExternalInput")
    wvT_d = nc.dram_tensor("wvT", [128, KO, 256], BF16, kind="ExternalInput")
    woT_d = nc.dram_tensor("woT", [128, 2, EMBED], BF16, kind="ExternalInput")
    bq_d = nc.dram_tensor("bq2", [128, 2], F32, kind="ExternalInput")
    bk_d = nc.dram_tensor("bk2", [128, 2], F32, kind="ExternalInput")
    bv_d = nc.dram_tensor("bvb", [128, 256], F32, kind="ExternalInput")
    mk_d = nc.dram_tensor("maskM", [128, 512], BF16, kind="ExternalInput")
    y_d = nc.dram_tensor("y", [S, EMBED], F32, kind="ExternalOutput")
    yv = y_d[:].rearrange("(so p) e -> so p e", p=128)

    with tile.TileContext(nc) as tc, ExitStack() as ctx:
        consts = ctx.enter_context(tc.tile_pool(name="consts", bufs=1))

        # weights first so the first projection matmuls can start early,
        # then x one contraction tile at a time in consumption order
        wqT = consts.tile([128, KO, 256], BF16, name="wqTs")
        nc.sync.dma_start(wqT, wqT_d[:])
        bq = consts.tile([128, 2], F32, name="bqs")
        nc.sync.dma_start(bq, bq_d[:])
        xT = consts.tile([128, KO, S], BF16, name="xTs")
        for k in range(KO):
            nc.sync.dma_start(xT[:, k, :], xT_d[:, k, :])
        wkT = consts.tile([128, KO, 256], BF16, name="wkTs")
        nc.sync.dma_start(wkT, wkT_d[:])
        bk = consts.tile([128, 2], F32, name="bks")
        nc.sync.dma_start(bk, bk_d[:])
        wvT = consts.tile([128, KO, 256], BF16, name="wvTs")
        nc.sync.dma_start(wvT, wvT_d[:])
        bvb = consts.tile([128, 256], F32, name="bvbs")
        nc.sync.dma_start(bvb, bv_d[:])
        maskM = consts.tile([128, 512], BF16, name="masks")
        nc.sync.dma_start(maskM, mk_d[:])
        woT = consts.tile([128, 2, EMBED], BF16, name="woTs")
        nc.sync.dma_start(woT, woT_d[:])

        # Persistent intermediates.
        # QT/KT: [d-part(64*eo+dl), hp, s]; Vp: [t-part, ttile, head, 64+ones]
        QT = consts.tile([128, 2, S], BF16, name="QTs")
        KT = consts.tile([128, 2, S], BF16, name="KTs")
        Vp = consts.tile([128, NTT, 4, 65], BF16, name="Vps")
        AO = consts.tile([128, 2, S], BF16, name="AOs")  # attnout^T per hp pair
        nc.vector.memset(Vp[:, :, :, 64:65], 1.0)

        # Prologue: Q/K projections for block 0, contraction-tile outer so
        # the matmuls track the xT input DMAs instead of waiting for all.
        with tc.tile_pool(name="prol_psum", bufs=1, space="PSUM") as prol:
            pq = [
                prol.tile([128, 512], F32, name=f"prol{i}", tag=f"prol{i}")
                for i in range(4)
            ]
            for k in range(KO):
                for i, (W, hp) in enumerate(((wqT, 0), (wqT, 1), (wkT, 0), (wkT, 1))):
                    nc.tensor.matmul(
                        pq[i],
                        W[:, k, 128 * hp : 128 * (hp + 1)],
                        xT[:, k, 0:512],
                        start=(k == 0),
                        stop=(k == KO - 1),
                    )
            for i, (OUT, bias_t, hp) in enumerate(
                ((QT, bq, 0), (QT, bq, 1), (KT, bk, 0), (KT, bk, 1))
            ):
                nc.vector.tensor_scalar_add(
                    OUT[:, hp, 0:512], pq[i], bias_t[:, hp : hp + 1]
                )

        # Single software-pipelined phase: Q/K/V projection psum-groups and the
        # out-projection are interleaved as PE filler inside the (ScalarE-paced)
        # attention k-loops, keeping the PE dense and HAM-warm.
        with tc.tile_pool(name="proj_psum", bufs=2, space="PSUM") as prp, tc.tile_pool(
            name="sc_psum", bufs=2, space="PSUM"
        ) as scp, tc.tile_pool(name="pvy_psum", bufs=2, space="PSUM") as pvyp, tc.tile_pool(
            name="ptp", bufs=2
        ) as ptp, tc.tile_pool(name="stg", bufs=3) as stg, tc.tile_pool(
            name="dscr", bufs=3, space="DRAM"
        ) as dscr:

            def emit_q_group(hp, sb):
                ps = prp.tile([128, 512], F32, name="prps", tag="prps")
                for k in range(KO):
                    nc.tensor.matmul(
                        ps,
                        wqT[:, k, 128 * hp : 128 * (hp + 1)],
                        xT[:, k, 512 * sb : 512 * (sb + 1)],
                        start=(k == 0),
                        stop=(k == KO - 1),
                    )
                nc.vector.tensor_scalar_add(
                    QT[:, hp, 512 * sb : 512 * (sb + 1)], ps, bq[:, hp : hp + 1]
                )

            def emit_k_group(hp, tb):
                ps = prp.tile([128, 512], F32, name="prps", tag="prps")
                for k in range(KO):
                    nc.tensor.matmul(
                        ps,
                        wkT[:, k, 128 * hp : 128 * (hp + 1)],
                        xT[:, k, 512 * tb : 512 * (tb + 1)],
                        start=(k == 0),
                        stop=(k == KO - 1),
                    )
                nc.vector.tensor_scalar_add(
                    KT[:, hp, 512 * tb : 512 * (tb + 1)], ps, bk[:, hp : hp + 1]
                )

            def emit_v_group(tt):
                ps = prp.tile([128, 512], F32, name="prps", tag="prps")
                for k in range(KO):
                    nc.tensor.matmul(
                        ps[:, 0:256],
                        xT[:, k, 128 * tt : 128 * (tt + 1)],
                        wvT[:, k, :],
                        start=(k == 0),
                        stop=(k == KO - 1),
                    )
                nc.vector.tensor_tensor(
                    Vp[:, tt, :, 0:64],
                    ps[:, 0:256].rearrange("p (h d) -> p h d", h=4),
                    bvb.rearrange("p (h d) -> p h d", h=4),
                    ALU.add,
                )

            def emit_y_group(st, eb):
                yt = prp.tile([128, 512], F32, name="yt", tag="prps")
                for hp in range(2):
                    nc.tensor.matmul(
                        yt,
                        AO[:, hp, 128 * st : 128 * (st + 1)],
                        woT[:, hp, 512 * eb : 512 * (eb + 1)],
                        start=(hp == 0),
                        stop=(hp == 1),
                    )
                ysb = stg.tile([128, 512], F32, name="ysb", tag="ysb")
                nc.vector.tensor_copy(ysb, yt)
                nc.sync.dma_start(yv[st][:, 512 * eb : 512 * (eb + 1)], ysb)

            LAG = 2
            for sb in range(NSB):
                # filler emitted during this sb's attention k-steps: this
                # block's V tiles, next block's Q/K, previous block's out-proj
                filler = []
                for tt in range(4 * sb, 4 * (sb + 1)):
                    filler.append(lambda tt=tt: emit_v_group(tt))
                if sb + 1 < NSB:
                    for hp in range(2):
                        filler.append(lambda hp=hp: emit_q_group(hp, sb + 1))
                        filler.append(lambda hp=hp: emit_k_group(hp, sb + 1))
                if sb >= 1:
                    for i in range(4):
                        for eb in range(2):
                            filler.append(
                                lambda st=4 * (sb - 1) + i, eb=eb: emit_y_group(st, eb)
                            )
                fi = iter(filler)

                for hp in range(2):
                    nt = 4 * sb + 4  # t-tiles (causal)
                    PT = ptp.tile([128, 2, NTT, 512], BF16, name="PT", tag="PT")
                    pvs = [
                        pvyp.tile([128, 512], F32, name=f"pv{eo}", tag="pvy")
                        for eo in range(2)
                    ]

                    def emit_pv(k, hp=hp, pvs=pvs, PT=PT, nt=nt):
                        r = k - 4 * sb
                        off = 128 * r if r > 0 else 0
                        for eo in range(2):
                            nc.tensor.matmul(
                                pvs[eo][0:65, off:512],
                                Vp[:, k, 2 * hp + eo, :],
                                PT[:, eo, k, off:512],
                                start=(k == 0),
                                stop=(k == nt - 1),
                            )

                    # scores + exp + (lagged) PV, interleaved per k-tile
                    for k in range(nt):
                        r = k - 4 * sb
                        off = 128 * r if r > 0 else 0
                        sc = scp.tile([128, 2, 512], F32, name="sc", tag="sc")
                        for eo in range(2):
                            nc.tensor.matmul(
                                sc[:, eo, off:512],
                                KT[64 * eo : 64 * eo + 64, hp, 128 * k : 128 * (k + 1)],
                                QT[
                                    64 * eo : 64 * eo + 64,
                                    hp,
                                    512 * sb + off : 512 * (sb + 1),
                                ],
                                start=True,
                                stop=True,
                            )
                        nc.scalar.activation(
                            PT[:, :, k, off:512],
                            sc[:, :, off:512],
                            AF.Exp,
                            bias=0.0,
                            scale=SCALE,
                        )
                        if r >= 0:  # diagonal: causal mask
                            for eo in range(2):
                                nc.vector.tensor_tensor(
                                    PT[:, eo, k, off:512],
                                    PT[:, eo, k, off:512],
                                    maskM[:, 0 : 512 - off],
                                    ALU.mult,
                                )
                        if k >= LAG:
                            emit_pv(k - LAG)
                        nxt = next(fi, None)
                        if nxt is not None:
                            nxt()
                    for k in range(max(0, nt - LAG), nt):
                        emit_pv(k)
                    # normalize: denom row -> SBUF -> DRAM -> broadcast [64,512],
                    # approx-reciprocal at base partition 0, multiply
                    rc = stg.tile([65, 2, 512], F32, name="rc", tag="rc")
                    scr = dscr.tile([2, 512], F32, name="scr", tag="scr")
                    for eo in range(2):
                        nc.vector.tensor_copy(rc[64:65, eo, :], pvs[eo][64:65, :])
                        nc.sync.dma_start(scr[eo : eo + 1, :], rc[64:65, eo, :])
                    for eo in range(2):
                        bc = stg.tile([64, 512], F32, name=f"bc{eo}", tag=f"bc{eo}")
                        nc.sync.dma_start(bc, scr[eo : eo + 1, :].to_broadcast((64, 512)))
                        rcp = stg.tile([64, 512], F32, name=f"rcp{eo}", tag=f"rcp{eo}")
                        nc.vector.reciprocal_approx_fast(rcp, bc)
                        if eo == 0:
                            nc.vector.tensor_tensor(
                                AO[0:64, hp, 512 * sb : 512 * (sb + 1)],
                                pvs[eo][0:64, :],
                                rcp,
                                ALU.mult,
                            )
                        else:
                            aos = stg.tile([64, 512], BF16, name="aos", tag="aos")
                            nc.vector.tensor_tensor(aos, pvs[eo][0:64, :], rcp, ALU.mult)
                            nc.sync.dma_start(
                                AO[64:128, hp, 512 * sb : 512 * (sb + 1)], aos
                            )
                # drain any remaining filler for this sb
                for nxt in fi:
                    nxt()
            for i in range(4):
                for eb in range(2):
                    emit_y_group(4 * (NSB - 1) + i, eb)
    nc.finalize()
    return nc


# ---------------------------------------------------------------------------


def _part3(a2d, n_inner=128):
    """[D0, D1] -> [128, D0//128, D1] with partition-inner grouping of dim 0."""
    d0, d1 = a2d.shape
    return np.ascontiguousarray(
        a2d.reshape(d0 // n_inner, n_inner, d1).transpose(1, 0, 2)
    )


def prep_core_inputs(x, Wq, bq, Wk, bk, Wv, bv, Wo, b, g, S):
    bf = ml_dtypes.bfloat16
    sl = slice(256 * g, 256 * (g + 1))
    xt = _part3(np.ascontiguousarray(x[b].T)).astype(bf)  # [128, KO, S]
    wqt = _part3(np.ascontiguousarray(Wq[sl, :].T)).astype(bf)  # [128, KO, 256]
    wkt = _part3(np.ascontiguousarray(Wk[sl, :].T)).astype(bf)
    wvt = _part3(np.ascontiguousarray(Wv[sl, :].T)).astype(bf)
    wot = _part3(np.ascontiguousarray(Wo[:, sl].T)).astype(bf)  # [128, 2, 1024]
    bq2 = np.ascontiguousarray(bq[sl].reshape(2, 128).T).astype(np.float32)
    bk2 = np.ascontiguousarray(bk[sl].reshape(2, 128).T).astype(np.float32)
    bvb = np.ascontiguousarray(np.broadcast_to(bv[sl], (128, 256))).astype(np.float32)
    mask = (np.arange(512)[None, :] >= np.arange(128)[:, None]).astype(bf)
    return {
        "xT": xt,
        "wqT": wqt,
        "wkT": wkt,
        "wvT": wvt,
        "woT": wot,
        "bq2": bq2,
        "bk2": bk2,
        "bvb": bvb,
        "maskM": mask,
    }


# ---------------------------------------------------------------------------
# PJRT runner: jit once, execute n_exec times (first execution after a fresh
# NEFF load has been observed to be flaky), return the last result.


def run_spmd(nc, in_maps, n_cores, n_exec=2):
    import jax
    from jax.sharding import Mesh, PartitionSpec
    from jax.experimental.shard_map import shard_map
    from concourse import bass2jax

    bass2jax.install_neuronx_cc_hook()

    partition_name = (
        nc.partition_id_tensor.name if nc.partition_id_tensor else None
    )
    in_names, out_names, out_avals, zero_outs = [], [], [], []
    for alloc in nc.m.functions[0].allocations:
        if not isinstance(alloc, mybir.MemoryLocationSet):
            continue
        name = alloc.memorylocations[0].name
        if alloc.kind == "ExternalInput":
            if name != partition_name:
                in_names.append(name)
        elif alloc.kind == "ExternalOutput":
            shape = tuple(alloc.tensor_shape)
            dtype = mybir.dt.np(alloc.dtype)
            out_names.append(name)
            out_avals.append(jax.core.ShapedArray(shape, dtype))
            zero_outs.append(np.zeros(shape, dtype))
    n_params = len(in_names)
    n_outs = len(out_avals)
    all_in_names = list(in_names) + list(out_names)
    if partition_name is not None:
        all_in_names.append(partition_name)
    donate = tuple(range(n_params, n_params + n_outs))

    def _body(*args):
        operands = list(args)
        if partition_name is not None:
            operands.append(bass2jax.partition_id_tensor())
        outs = bass2jax._bass_exec_p.bind(
            *operands,
            out_avals=tuple(out_avals),
            in_names=tuple(all_in_names),
            out_names=tuple(out_names),
            lowering_input_output_aliases=(),
            sim_require_finite=True,
            sim_require_nnan=True,
            nc=nc,
        )
        return tuple(outs)

    devices = jax.devices()[:n_cores]
    mesh = Mesh(np.asarray(devices), ("core",))
    sharded = jax.jit(
        shard_map(
            _body,
            mesh=mesh,
            in_specs=(PartitionSpec("core"),) * (n_params + n_outs),
            out_specs=(PartitionSpec("core"),) * n_outs,
            check_rep=False,
        ),
        donate_argnums=donate,
        keep_unused=True,
    )
    per_core = [[np.asarray(m[name]) for name in in_names] for m in in_maps]
    concat_in = [
        np.concatenate([per_core[c][i] for c in range(n_cores)], axis=0)
        for i in range(n_params)
    ]
    out_arrs = None
    for _ in range(max(1, n_exec)):
        concat_zeros = [
            np.zeros((n_cores * z.shape[0], *z.shape[1:]), z.dtype) for z in zero_outs
        ]
        out_arrs = sharded(*concat_in, *concat_zeros)
    return [
        {
            name: np.asarray(out_arrs[i]).reshape(n_cores, *out_avals[i].shape)[c]
            for i, name in enumerate(out_names)
        }
        for c in range(n_cores)
    ]


_NC_CACHE = {}


def kernel(x, Wq, bq, Wk, bk, Wv, bv, Wo, bo):
    x = np.asarray(x, dtype=np.float32)
    Wq = np.asarray(Wq, dtype=np.float32)
    bq = np.asarray(bq, dtype=np.float32)
    Wk = np.asarray(Wk, dtype=np.float32)
    bk = np.asarray(bk, dtype=np.float32)
    Wv = np.asarray(Wv, dtype=np.float32)
    bv = np.asarray(bv, dtype=np.float32)
    Wo = np.asarray(Wo, dtype=np.float32)
    bo = np.asarray(bo, dtype=np.float32)

    B, S, E = x.shape
    assert E == EMBED
    if S not in _NC_CACHE:
        _NC_CACHE[S] = build_nc(S)
    nc = _NC_CACHE[S]

    in_maps = [
        prep_core_inputs(x, Wq, bq, Wk, bk, Wv, bv, Wo, c // 4, c % 4, S)
        for c in range(N_CORES)
    ]
    results = run_spmd(nc, in_maps, N_CORES)
    out = np.empty((B, S, E), dtype=np.float32)
    for b in range(B):
        acc = results[4 * b]["y"].astype(np.float32).copy()
        for g in range(1, 4):
            acc += results[4 * b + g]["y"]
        out[b] = acc + bo[None, :]
    return out
